# revision 2
# baseline (speedup 1.0000x reference)
"""Trainium2 Bass kernel for nn_AutoregressiveBeamDecoder.

Data-parallel over batch: 8 cores x 32 rows. Per step (T=128, sequential due
to argmax feedback): embedding gather via one-hot matmul, GRU cell GEMMs in
fp32 (exact), LayerNorm, output GEMMs, argmax -> one-hot for the next step.
ctx-dependent GEMM halves (A_t = ctx_t @ W_in1.T, C_t = ctx_t @ W_o1b.T) are
hoisted out of the loop; one-time init (h0, E2 table) is host-prepped.
"""
import sys

sys.path.insert(0, "/opt/trn_rl_repo")
import numpy as np

B, T, D, H, NB, HH = 256, 128, 512, 1024, 64, 8
NC = 8
BL = B // NC  # 32 rows per core
KT = H // 128  # 8 k-tiles
LN_EPS = 1e-5


def _build():
    import concourse.bass as bass
    import concourse.tile as tile
    from concourse import bacc, mybir
    from concourse.bass import ds
    from concourse.masks import make_identity

    f32 = mybir.dt.float32
    nc = bacc.Bacc("TRN2", target_bir_lowering=False, debug=False, num_devices=NC)

    at_d = nc.dram_tensor("at", (T, 128, KT, BL), f32, kind="ExternalInput")
    c_d = nc.dram_tensor("cmat", (T, BL, H), f32, kind="ExternalInput")
    wih_d = nc.dram_tensor("wih", (6, 128, KT, 512), f32, kind="ExternalInput")
    whh_d = nc.dram_tensor("whh", (6, 128, KT, 512), f32, kind="ExternalInput")
    wo1_d = nc.dram_tensor("wo1", (128, KT, H), f32, kind="ExternalInput")
    wo2_d = nc.dram_tensor("wo2", (128, KT, NB), f32, kind="ExternalInput")
    e2_d = nc.dram_tensor("e2", (NB, H), f32, kind="ExternalInput")
    h0_d = nc.dram_tensor("h0", (BL, H), f32, kind="ExternalInput")
    h0t_d = nc.dram_tensor("h0t", (128, KT, BL), f32, kind="ExternalInput")
    oh0_d = nc.dram_tensor("oh0", (NB, BL), f32, kind="ExternalInput")
    brz_d = nc.dram_tensor("brz", (BL, 2 * H), f32, kind="ExternalInput")
    bxn_d = nc.dram_tensor("bxn", (BL, H), f32, kind="ExternalInput")
    bhn_d = nc.dram_tensor("bhn", (BL, H), f32, kind="ExternalInput")
    bo2_d = nc.dram_tensor("bo2", (BL, NB), f32, kind="ExternalInput")
    g_d = nc.dram_tensor("lng", (BL, H), f32, kind="ExternalInput")
    bb_d = nc.dram_tensor("lnb", (BL, H), f32, kind="ExternalInput")
    out_d = nc.dram_tensor("outp", (T, BL, NB), f32, kind="ExternalOutput")

    with tile.TileContext(nc) as tc:
        with (
            tc.tile_pool(name="singles", bufs=1) as sg,
            tc.tile_pool(name="wpool", bufs=2) as wp,
            tc.tile_pool(name="work", bufs=1) as wk,
            tc.tile_pool(name="pg", bufs=2, space="PSUM") as pg,
            tc.tile_pool(name="pmisc", bufs=1, space="PSUM") as pm,
        ):
            # ---- resident constants / state ----
            wo1_sb = sg.tile([128, KT, H], f32)
            nc.sync.dma_start(out=wo1_sb, in_=wo1_d[:])
            wo2_sb = sg.tile([128, KT, NB], f32)
            nc.sync.dma_start(out=wo2_sb, in_=wo2_d[:])
            e2_sb = sg.tile([NB, H], f32)
            nc.sync.dma_start(out=e2_sb, in_=e2_d[:])
            brz_sb = sg.tile([BL, 2 * H], f32)
            nc.sync.dma_start(out=brz_sb, in_=brz_d[:])
            bxn_sb = sg.tile([BL, H], f32)
            nc.sync.dma_start(out=bxn_sb, in_=bxn_d[:])
            bhn_sb = sg.tile([BL, H], f32)
            nc.sync.dma_start(out=bhn_sb, in_=bhn_d[:])
            bo2_sb = sg.tile([BL, NB], f32)
            nc.sync.dma_start(out=bo2_sb, in_=bo2_d[:])
            g_sb = sg.tile([BL, H], f32)
            nc.sync.dma_start(out=g_sb, in_=g_d[:])
            bb_sb = sg.tile([BL, H], f32)
            nc.sync.dma_start(out=bb_sb, in_=bb_d[:])
            ident = sg.tile([BL, BL], f32)
            make_identity(nc, ident)
            eps_sb = sg.tile([BL, 1], f32)
            nc.vector.memset(eps_sb, LN_EPS)

            h_sb = sg.tile([BL, H], f32)
            nc.sync.dma_start(out=h_sb, in_=h0_d[:])
            ht_sb = sg.tile([128, KT, BL], f32)
            nc.sync.dma_start(out=ht_sb, in_=h0t_d[:])
            oht_sb = sg.tile([NB, BL], f32)
            nc.sync.dma_start(out=oht_sb, in_=oh0_d[:])

            with tc.For_i(0, T, 1) as t:
                at_sb = wk.tile([128, KT, BL], f32)
                nc.sync.dma_start(out=at_sb, in_=at_d[ds(t, 1)][0])
                c_sb = wk.tile([BL, H], f32)
                nc.sync.dma_start(out=c_sb, in_=c_d[ds(t, 1)][0])

                # ---- x^T = relu(A_t^T + E2^T[:, prev]) ----
                gps = pm.tile([128, KT, BL], f32, tag="gather")
                for k in range(KT):
                    nc.tensor.matmul(
                        gps[:, k], e2_sb[:, ds(128 * k, 128)], oht_sb
                    )
                xt_sb = wk.tile([128, KT, BL], f32)
                nc.vector.tensor_add(xt_sb, gps, at_sb)
                nc.vector.tensor_scalar_max(xt_sb, xt_sb, 0.0)

                # ---- r,z gates: 4 chunks of 512 over [x@Wih + h@Whh] ----
                rz_sb = wk.tile([BL, 2 * H], f32)
                for c in range(4):
                    wih_sb = wp.tile([128, KT, 512], f32, tag="wih")
                    nc.sync.dma_start(out=wih_sb, in_=wih_d[c])
                    whh_sb = wp.tile([128, KT, 512], f32, tag="whh")
                    nc.sync.dma_start(out=whh_sb, in_=whh_d[c])
                    ps = pg.tile([BL, 512], f32, tag="gemm")
                    for k in range(KT):
                        nc.tensor.matmul(
                            ps, xt_sb[:, k], wih_sb[:, k], start=(k == 0), stop=False
                        )
                    for k in range(KT):
                        nc.tensor.matmul(
                            ps, ht_sb[:, k], whh_sb[:, k], start=False, stop=(k == KT - 1)
                        )
                    nc.vector.tensor_copy(rz_sb[:, ds(512 * c, 512)], ps)
                nc.vector.tensor_add(rz_sb, rz_sb, brz_sb)
                nc.scalar.activation(
                    rz_sb, rz_sb, mybir.ActivationFunctionType.Sigmoid
                )

                # ---- xn, hn ----
                xn_sb = wk.tile([BL, H], f32)
                hn_sb = wk.tile([BL, H], f32)
                for c in range(2):
                    wih_sb = wp.tile([128, KT, 512], f32, tag="wih")
                    nc.sync.dma_start(out=wih_sb, in_=wih_d[4 + c])
                    ps = pg.tile([BL, 512], f32, tag="gemm")
                    for k in range(KT):
                        nc.tensor.matmul(
                            ps, xt_sb[:, k], wih_sb[:, k], start=(k == 0), stop=(k == KT - 1)
                        )
                    nc.vector.tensor_copy(xn_sb[:, ds(512 * c, 512)], ps)
                for c in range(2):
                    whh_sb = wp.tile([128, KT, 512], f32, tag="whh")
                    nc.sync.dma_start(out=whh_sb, in_=whh_d[4 + c])
                    ps = pg.tile([BL, 512], f32, tag="gemm")
                    for k in range(KT):
                        nc.tensor.matmul(
                            ps, ht_sb[:, k], whh_sb[:, k], start=(k == 0), stop=(k == KT - 1)
                        )
                    nc.vector.tensor_copy(hn_sb[:, ds(512 * c, 512)], ps)
                nc.vector.tensor_add(xn_sb, xn_sb, bxn_sb)
                nc.vector.tensor_add(hn_sb, hn_sb, bhn_sb)

                # ---- n = tanh(xn + r*hn); h = n + z*(h - n) ----
                nc.vector.tensor_mul(hn_sb, rz_sb[:, 0:H], hn_sb)
                nc.vector.tensor_add(hn_sb, hn_sb, xn_sb)
                nc.scalar.activation(
                    hn_sb, hn_sb, mybir.ActivationFunctionType.Tanh
                )  # hn_sb = n
                tmp_sb = wk.tile([BL, H], f32)
                nc.vector.tensor_sub(tmp_sb, h_sb, hn_sb)
                nc.vector.tensor_mul(tmp_sb, rz_sb[:, H : 2 * H], tmp_sb)
                nc.vector.tensor_add(h_sb, hn_sb, tmp_sb)  # h_new

                # ---- transpose h_new -> ht_sb (for next step + not needed now) ----
                tps = pm.tile([128, KT, BL], f32, tag="tr")
                for k in range(KT):
                    nc.tensor.transpose(tps[:, k], h_sb[:, ds(128 * k, 128)], ident)
                nc.vector.tensor_copy(ht_sb, tps)

                # ---- layernorm(h_new) ----
                stats = wk.tile([BL, 2, 6], f32)
                hv = h_sb.rearrange("p (s q) -> p s q", s=2)
                for s in range(2):
                    nc.vector.bn_stats(out=stats[:, s], in_=hv[:, s])
                mv = wk.tile([BL, 2], f32)
                nc.vector.bn_aggr(out=mv, in_=stats)
                rstd = wk.tile([BL, 1], f32)
                nc.scalar.activation(
                    rstd,
                    mv[:, 1:2],
                    mybir.ActivationFunctionType.Sqrt,
                    bias=eps_sb,
                    scale=1.0,
                )
                nc.vector.reciprocal(rstd, rstd)
                hnorm_sb = wk.tile([BL, H], f32)
                nc.vector.tensor_scalar(
                    out=hnorm_sb,
                    in0=h_sb,
                    scalar1=mv[:, 0:1],
                    scalar2=rstd,
                    op0=mybir.AluOpType.subtract,
                    op1=mybir.AluOpType.mult,
                )
                nc.vector.tensor_mul(hnorm_sb, hnorm_sb, g_sb)
                nc.vector.tensor_add(hnorm_sb, hnorm_sb, bb_sb)

                # ---- transpose hnorm; o = relu(hnorm @ Wo1a.T + C_t) ----
                tps2 = pm.tile([128, KT, BL], f32, tag="tr")
                for k in range(KT):
                    nc.tensor.transpose(tps2[:, k], hnorm_sb[:, ds(128 * k, 128)], ident)
                hnt_sb = wk.tile([128, KT, BL], f32)
                nc.vector.tensor_copy(hnt_sb, tps2)
                o_sb = wk.tile([BL, H], f32)
                for c in range(2):
                    ps = pg.tile([BL, 512], f32, tag="gemm")
                    for k in range(KT):
                        nc.tensor.matmul(
                            ps,
                            hnt_sb[:, k],
                            wo1_sb[:, k, ds(512 * c, 512)],
                            start=(k == 0),
                            stop=(k == KT - 1),
                        )
                    nc.vector.tensor_add(o_sb[:, ds(512 * c, 512)], ps, c_sb[:, ds(512 * c, 512)])
                nc.vector.tensor_scalar_max(o_sb, o_sb, 0.0)

                # ---- transpose o; logits = o @ Wo2.T + b_o2 ----
                tps3 = pm.tile([128, KT, BL], f32, tag="tr")
                for k in range(KT):
                    nc.tensor.transpose(tps3[:, k], o_sb[:, ds(128 * k, 128)], ident)
                ot_sb = wk.tile([128, KT, BL], f32)
                nc.vector.tensor_copy(ot_sb, tps3)
                psl = pm.tile([BL, NB], f32, tag="lg")
                for k in range(KT):
                    nc.tensor.matmul(
                        psl, ot_sb[:, k], wo2_sb[:, k], start=(k == 0), stop=(k == KT - 1)
                    )
                lg_sb = wk.tile([BL, NB], f32)
                nc.vector.tensor_add(lg_sb, psl, bo2_sb)
                nc.sync.dma_start(out=out_d[ds(t, 1)][0], in_=lg_sb)

                # ---- argmax -> one-hot^T for next step ----
                mx = wk.tile([BL, 1], f32)
                nc.vector.tensor_reduce(
                    out=mx, in_=lg_sb, axis=mybir.AxisListType.X, op=mybir.AluOpType.max
                )
                oh_sb = wk.tile([BL, NB], f32)
                nc.vector.tensor_scalar(
                    out=oh_sb,
                    in0=lg_sb,
                    scalar1=mx,
                    scalar2=None,
                    op0=mybir.AluOpType.is_ge,
                )
                pso = pm.tile([NB, BL], f32, tag="oh")
                nc.tensor.transpose(pso, oh_sb, ident)
                nc.vector.tensor_copy(oht_sb, pso)

    nc.compile()
    return nc


def _prep_core(I, core):
    """Host-side layout prep for one core's shard (batch rows 32c..32c+32)."""
    sl = slice(core * BL, (core + 1) * BL)
    cf = np.asarray(I["context_features"], np.float32)[sl]  # (32,T,512)
    bh = np.asarray(I["beam_history"])[sl].astype(np.int64)
    be = np.asarray(I["beam_embed"], np.float32)
    W_in = np.asarray(I["W_in"], np.float32)
    b_in = np.asarray(I["b_in"], np.float32)
    W_init = np.asarray(I["W_init"], np.float32)
    b_init = np.asarray(I["b_init"], np.float32)
    W_ih = np.asarray(I["W_ih"], np.float32)
    b_ih = np.asarray(I["b_ih"], np.float32)
    W_hh = np.asarray(I["W_hh"], np.float32)
    b_hh = np.asarray(I["b_hh"], np.float32)
    W_o1 = np.asarray(I["W_o1"], np.float32)
    b_o1 = np.asarray(I["b_o1"], np.float32)
    W_o2 = np.asarray(I["W_o2"], np.float32)
    b_o2 = np.asarray(I["b_o2"], np.float32)
    ln_g = np.asarray(I["ln_g"], np.float32)
    ln_b = np.asarray(I["ln_b"], np.float32)

    # hoisted ctx GEMMs (layout/packing prep)
    A = cf @ W_in[:, :D].T  # (32,T,H)
    C = cf @ W_o1[:, H:].T + b_o1  # (32,T,H)
    at = np.ascontiguousarray(
        A.transpose(1, 2, 0).reshape(T, KT, 128, BL).transpose(0, 2, 1, 3)
    )  # (T,128,KT,32)
    cmat = np.ascontiguousarray(C.transpose(1, 0, 2))  # (T,32,H)

    # one-time init on host
    prev0 = bh[:, -1]
    hist = be[bh].mean(1)
    ctxg = cf.mean(1)
    h0 = np.tanh(np.concatenate([ctxg, hist], -1) @ W_init.T + b_init).astype(np.float32)
    h0t = np.ascontiguousarray(h0.T.reshape(KT, 128, BL).transpose(1, 0, 2))
    oh0 = np.zeros((NB, BL), np.float32)
    oh0[prev0, np.arange(BL)] = 1.0
    e2 = (be @ W_in[:, D:].T + b_in).astype(np.float32)

    def chunks6(w):  # (3H,H) -> (6,128,KT,512) of w.T column chunks
        wt = np.ascontiguousarray(w.T)  # (H,3H)
        return np.ascontiguousarray(
            wt.reshape(KT, 128, 6, 512).transpose(2, 1, 0, 3)
        )

    wo1 = np.ascontiguousarray(
        W_o1[:, :H].T.reshape(KT, 128, H).transpose(1, 0, 2)
    )
    wo2 = np.ascontiguousarray(W_o2.T.reshape(KT, 128, NB).transpose(1, 0, 2))

    bc = lambda v, n: np.ascontiguousarray(np.broadcast_to(v, (BL, n)).astype(np.float32))
    return {
        "at": at,
        "cmat": cmat,
        "wih": chunks6(W_ih),
        "whh": chunks6(W_hh),
        "wo1": wo1,
        "wo2": wo2,
        "e2": e2,
        "h0": h0,
        "h0t": h0t,
        "oh0": oh0,
        "brz": bc(b_ih[: 2 * H] + b_hh[: 2 * H], 2 * H),
        "bxn": bc(b_ih[2 * H :], H),
        "bhn": bc(b_hh[2 * H :], H),
        "bo2": bc(b_o2, NB),
        "lng": bc(ln_g, H),
        "lnb": bc(ln_b, H),
    }


def _unshard(res):
    out = np.zeros((B, T, NB), np.float32)
    for c in range(NC):
        out[c * BL : (c + 1) * BL] = res.results[c]["outp"].transpose(1, 0, 2)
    return out


def kernel(**inputs) -> np.ndarray:
    from concourse import bass_utils

    nc = _build()
    in_maps = [_prep_core(inputs, c) for c in range(NC)]
    res = bass_utils.run_bass_kernel_spmd(nc, in_maps, core_ids=list(range(NC)))
    return _unshard(res)


if __name__ == "__main__":
    import reference as R

    I = {k: np.asarray(v) for k, v in R.setup_inputs().items()}
    got = kernel(**I)
    import jax.numpy as jnp

    exp = np.asarray(R.reference(**{k: jnp.asarray(v) for k, v in I.items()}))
    d = np.abs(got - exp)
    print("absmax err:", d.max())
    print("Relative error:", np.linalg.norm(got - exp) / np.linalg.norm(exp))



# revision 6
# speedup vs baseline: 6.6589x; 6.6589x over previous
"""Trainium2 Bass kernel for nn_AutoregressiveBeamDecoder.

Data-parallel over batch: 8 cores x 32 rows. Per step (T=128, sequential due
to argmax feedback): embedding gather via one-hot matmul, GRU cell GEMMs in
bf16 (weights SBUF-resident), folded LayerNorm, output GEMMs, argmax.

Layout: H-sized vectors are packed (128, 256): partition 32q+b holds batch
row b's H-quarter q. Gate GEMMs write 4 PSUM partition strips concurrently
via col-group tile_position. LN stats use fused reduce + two tiny
constant-matrix matmuls (quarter-combine cb, quarter-broadcast bb).
ctx-dependent GEMM halves (A_t = ctx_t @ W_in1.T, C_t = ctx_t @ W_o1b.T +
b_o1 + ln_b @ W_o1a.T) are hoisted to the host; h-side gate GEMMs are issued
at the top of each step so the PE overlaps the previous step's tail.
"""
import sys

sys.path.insert(0, "/opt/trn_rl_repo")
import numpy as np
import ml_dtypes

BF16 = ml_dtypes.bfloat16
B, T, D, H, NB, HH = 256, 128, 512, 1024, 64, 8
NC = 8
BL = B // NC  # 32 rows per core
KT = H // 128  # 8 k-tiles
Q = 256  # H quarter
LN_EPS = 1e-5


def _build():
    import concourse.bass as bass
    import concourse.tile as tile
    from concourse import bacc, mybir
    from concourse.bass import ds

    f32 = mybir.dt.float32
    bf16 = mybir.dt.bfloat16
    nc = bacc.Bacc("TRN2", target_bir_lowering=False, debug=False, num_devices=NC)

    # weights / constants (host-packed)
    wrz_ih_d = nc.dram_tensor("wrz_ih", (128, KT, 4, 2 * Q), bf16, kind="ExternalInput")
    wrz_hh_d = nc.dram_tensor("wrz_hh", (128, KT, 4, 2 * Q), bf16, kind="ExternalInput")
    wn_ih_d = nc.dram_tensor("wn_ih", (128, KT, 4, Q), bf16, kind="ExternalInput")
    wn_hh_d = nc.dram_tensor("wn_hh", (128, KT, 4, Q), bf16, kind="ExternalInput")
    wo1_d = nc.dram_tensor("wo1", (128, KT, 4, Q), bf16, kind="ExternalInput")
    wo2_d = nc.dram_tensor("wo2", (128, KT, NB), bf16, kind="ExternalInput")
    e2_d = nc.dram_tensor("e2", (NB, H), bf16, kind="ExternalInput")
    u_d = nc.dram_tensor("upk", (128, Q), f32, kind="ExternalInput")
    brz_d = nc.dram_tensor("brz", (128, 2 * Q), f32, kind="ExternalInput")
    bxn_d = nc.dram_tensor("bxn", (128, Q), f32, kind="ExternalInput")
    bhn_d = nc.dram_tensor("bhn", (128, Q), f32, kind="ExternalInput")
    bo2_d = nc.dram_tensor("bo2", (BL, NB), f32, kind="ExternalInput")
    id128_d = nc.dram_tensor("id128", (128, 128), f32, kind="ExternalInput")
    cb_d = nc.dram_tensor("cb", (128, 32), f32, kind="ExternalInput")
    bb_d = nc.dram_tensor("bb", (32, 128), f32, kind="ExternalInput")
    # state init
    h0_d = nc.dram_tensor("h0pk", (128, Q), f32, kind="ExternalInput")
    h0t_d = nc.dram_tensor("h0t", (128, 2 * 128), bf16, kind="ExternalInput")
    oh0_d = nc.dram_tensor("oh0", (NB, BL), bf16, kind="ExternalInput")
    # per-step streams
    at_d = nc.dram_tensor("at", (T, 128, KT * BL), f32, kind="ExternalInput")
    c_d = nc.dram_tensor("cpk", (T, 128, Q), f32, kind="ExternalInput")
    out_d = nc.dram_tensor("outp", (T, BL, NB), f32, kind="ExternalOutput")

    with tile.TileContext(nc) as tc:
        with (
            tc.tile_pool(name="singles", bufs=1) as sg,
            tc.tile_pool(name="work", bufs=2) as wk,
            tc.tile_pool(name="pp", bufs=1, space="PSUM") as pp,
        ):
            # resident weights / constants
            wrz_ih = sg.tile([128, KT, 4, 2 * Q], bf16)
            nc.sync.dma_start(out=wrz_ih, in_=wrz_ih_d[:])
            wrz_hh = sg.tile([128, KT, 4, 2 * Q], bf16)
            nc.sync.dma_start(out=wrz_hh, in_=wrz_hh_d[:])
            wn_ih = sg.tile([128, KT, 4, Q], bf16)
            nc.sync.dma_start(out=wn_ih, in_=wn_ih_d[:])
            wn_hh = sg.tile([128, KT, 4, Q], bf16)
            nc.sync.dma_start(out=wn_hh, in_=wn_hh_d[:])
            wo1 = sg.tile([128, KT, 4, Q], bf16)
            nc.sync.dma_start(out=wo1, in_=wo1_d[:])
            wo2 = sg.tile([128, KT, NB], bf16)
            nc.sync.dma_start(out=wo2, in_=wo2_d[:])
            e2 = sg.tile([NB, H], bf16)
            nc.sync.dma_start(out=e2, in_=e2_d[:])
            u_pk = sg.tile([128, Q], f32)
            nc.sync.dma_start(out=u_pk, in_=u_d[:])
            brz = sg.tile([128, 2 * Q], f32)
            nc.sync.dma_start(out=brz, in_=brz_d[:])
            bxn = sg.tile([128, Q], f32)
            nc.sync.dma_start(out=bxn, in_=bxn_d[:])
            bhn = sg.tile([128, Q], f32)
            nc.sync.dma_start(out=bhn, in_=bhn_d[:])
            bo2 = sg.tile([BL, NB], f32)
            nc.sync.dma_start(out=bo2, in_=bo2_d[:])
            id128 = sg.tile([128, 128], f32)
            nc.sync.dma_start(out=id128, in_=id128_d[:])
            cb = sg.tile([128, 32], f32)
            nc.sync.dma_start(out=cb, in_=cb_d[:])
            bb = sg.tile([32, 128], f32)
            nc.sync.dma_start(out=bb, in_=bb_d[:])
            eps = sg.tile([BL, 1], f32)
            nc.vector.memset(eps, LN_EPS)

            # state
            h_pk = sg.tile([128, Q], f32)
            nc.sync.dma_start(out=h_pk, in_=h0_d[:])
            ht = sg.tile([128, 2 * 128], bf16)
            nc.sync.dma_start(out=ht, in_=h0t_d[:])
            oht = sg.tile([NB, BL], bf16)
            nc.sync.dma_start(out=oht, in_=oh0_d[:])

            # psum tiles (persistent; accumulation groups via start/stop)
            RZ = pp.tile([128, 2 * Q], f32, tag="rz")
            NX = pp.tile([128, 2 * Q], f32, tag="nx")  # [0:Q] hn, [Q:2Q] xn
            OO = pp.tile([128, Q], f32, tag="oo")
            M1 = pp.tile([128, 512], f32, tag="m1")  # [0:256] ht-tr, [256:512] gather
            M2 = pp.tile([128, 512], f32, tag="m2")  # [0:256] o-tr
            SM = pp.tile([128, 128], f32, tag="sm")  # cs/bc/LG/OHT small psums
            cs_ps = SM[0:32, 0:2]
            bc_ps = SM[:, 2:4]
            lg_ps = SM[0:32, 4:68]
            oht_ps = SM[0:64, 68:100]

            kts = lambda k: ds(128 * (k % 2) + 32 * (k // 2), 32)

            with tc.For_i(0, T, 1) as t:
                at_sb = wk.tile([128, KT * BL], f32, tag="at")
                nc.sync.dma_start(out=at_sb, in_=at_d[ds(t, 1)][0])
                c_sb = wk.tile([128, Q], f32, tag="c")
                nc.sync.dma_start(out=c_sb, in_=c_d[ds(t, 1)][0])

                # ---- h-side gate GEMMs (only need ht from prev step) ----
                for k in range(KT):
                    for q in range(4):
                        nc.tensor.matmul(
                            RZ[32 * q : 32 * q + 32, :], ht[:, kts(k)], wrz_hh[:, k, q],
                            start=(k == 0), stop=False, tile_position=(0, 32 * q),
                        )
                    for q in range(4):
                        nc.tensor.matmul(
                            NX[32 * q : 32 * q + 32, 0:Q], ht[:, kts(k)], wn_hh[:, k, q],
                            start=(k == 0), stop=(k == KT - 1), tile_position=(0, 32 * q),
                        )

                # ---- gather x^T = E2^T[:, prev] ----
                for k in range(KT):
                    nc.tensor.matmul(
                        M1[:, ds(256 + 32 * k, 32)], e2[:, ds(128 * k, 128)], oht,
                        start=True, stop=True,
                    )
                xf = wk.tile([128, KT * BL], f32, tag="xf")
                nc.vector.tensor_add(xf, M1[:, ds(256, 256)], at_sb)
                xt = wk.tile([128, KT * BL], bf16, tag="xt")
                nc.vector.tensor_scalar_max(xt, xf, 0.0)

                # ---- x-side gate GEMMs ----
                for k in range(KT):
                    for q in range(4):
                        nc.tensor.matmul(
                            RZ[32 * q : 32 * q + 32, :], xt[:, ds(32 * k, 32)], wrz_ih[:, k, q],
                            start=False, stop=(k == KT - 1), tile_position=(0, 32 * q),
                        )
                    for q in range(4):
                        nc.tensor.matmul(
                            NX[32 * q : 32 * q + 32, Q : 2 * Q], xt[:, ds(32 * k, 32)], wn_ih[:, k, q],
                            start=(k == 0), stop=(k == KT - 1), tile_position=(0, 32 * q),
                        )

                # ---- GRU cell (packed layout, 128 partitions) ----
                rzb = wk.tile([128, 2 * Q], f32, tag="rzb")
                nc.vector.tensor_add(rzb, RZ, brz)
                rs = wk.tile([128, 2 * Q], f32, tag="rs")
                nc.scalar.activation(rs, rzb, mybir.ActivationFunctionType.Sigmoid)
                hnb = wk.tile([128, Q], f32, tag="hnb")
                nc.vector.tensor_add(hnb, NX[:, 0:Q], bhn)
                xnb = wk.tile([128, Q], f32, tag="xnb")
                nc.vector.tensor_add(xnb, NX[:, Q : 2 * Q], bxn)
                un = wk.tile([128, Q], f32, tag="un")
                nc.vector.tensor_mul(un, rs[:, 0:Q], hnb)
                nc.vector.tensor_add(un, un, xnb)
                nt = wk.tile([128, Q], f32, tag="nt")
                nc.scalar.activation(nt, un, mybir.ActivationFunctionType.Tanh)
                dd = wk.tile([128, Q], f32, tag="dd")
                nc.vector.tensor_sub(dd, h_pk, nt)
                zd = wk.tile([128, Q], f32, tag="zd")
                nc.vector.tensor_mul(zd, rs[:, Q : 2 * Q], dd)
                st = wk.tile([128, 2], f32, tag="st")
                nc.vector.tensor_add(h_pk, nt, zd)
                nc.vector.tensor_reduce(
                    out=st[:, 0:1], in_=h_pk, axis=mybir.AxisListType.X,
                    op=mybir.AluOpType.add,
                )
                hsq = wk.tile([128, Q], f32, tag="hsq")
                nc.scalar.activation(
                    hsq, h_pk, mybir.ActivationFunctionType.Square,
                    accum_out=st[:, 1:2],
                )

                # ---- transpose h -> ht (row-group concurrent) ----
                nc.tensor.transpose(M1[:, ds(0, 128)], h_pk[:, ds(0, 128)], id128)
                nc.tensor.transpose(M1[:, ds(128, 128)], h_pk[:, ds(128, 128)], id128)
                nc.vector.tensor_copy(ht, M1[:, ds(0, 256)])

                # ---- o1 GEMMs (use new ht; LN folded into wo1/u/c) ----
                for k in range(KT):
                    for q in range(4):
                        nc.tensor.matmul(
                            OO[32 * q : 32 * q + 32, :], ht[:, kts(k)], wo1[:, k, q],
                            start=(k == 0), stop=(k == KT - 1), tile_position=(0, 32 * q),
                        )

                # ---- LN stats: quarter-combine + broadcast via tiny matmuls ----
                nc.tensor.matmul(cs_ps, cb, st, start=True, stop=True)
                m32 = wk.tile([BL, 4], f32, tag="m32")
                nc.vector.tensor_scalar_mul(m32[:, 0:1], cs_ps[:, 0:1], 1.0 / H)
                nc.vector.tensor_scalar_mul(m32[:, 1:2], cs_ps[:, 1:2], 1.0 / H)
                nc.vector.tensor_mul(m32[:, 2:3], m32[:, 0:1], m32[:, 0:1])
                nc.vector.tensor_sub(m32[:, 3:4], m32[:, 1:2], m32[:, 2:3])
                sd32 = wk.tile([BL, 1], f32, tag="sd32")
                nc.scalar.activation(
                    sd32, m32[:, 3:4], mybir.ActivationFunctionType.Sqrt,
                    bias=eps, scale=1.0,
                )
                br = wk.tile([BL, 2], f32, tag="br")
                nc.vector.reciprocal(br[:, 0:1], sd32)  # rstd
                nc.vector.tensor_mul(br[:, 1:2], m32[:, 0:1], br[:, 0:1])  # m*rstd
                nc.tensor.matmul(bc_ps, bb, br, start=True, stop=True)
                bc = wk.tile([128, 2], f32, tag="bc")
                nc.vector.tensor_copy(bc, bc_ps)

                # ---- o = relu(OO*rstd - (m*rstd)*u + C') ----
                o1s = wk.tile([128, Q], f32, tag="o1s")
                nc.vector.tensor_scalar(
                    out=o1s, in0=OO, scalar1=bc[:, 0:1], scalar2=None,
                    op0=mybir.AluOpType.mult,
                )
                mu = wk.tile([128, Q], f32, tag="mu")
                nc.vector.tensor_scalar(
                    out=mu, in0=u_pk, scalar1=bc[:, 1:2], scalar2=None,
                    op0=mybir.AluOpType.mult,
                )
                nc.vector.tensor_sub(mu, c_sb, mu)
                op = wk.tile([128, Q], f32, tag="op")
                nc.vector.tensor_add(op, o1s, mu)
                nc.vector.tensor_scalar_max(op, op, 0.0)

                # ---- transpose o -> ot; logits ----
                nc.tensor.transpose(M2[:, ds(0, 128)], op[:, ds(0, 128)], id128)
                nc.tensor.transpose(M2[:, ds(128, 128)], op[:, ds(128, 128)], id128)
                ot = wk.tile([128, 2 * 128], bf16, tag="ot")
                nc.vector.tensor_copy(ot, M2[:, ds(0, 256)])
                for k in range(KT):
                    nc.tensor.matmul(
                        lg_ps, ot[:, kts(k)], wo2[:, k], start=(k == 0), stop=(k == KT - 1)
                    )
                lgb = wk.tile([BL, NB], f32, tag="lgb")
                nc.vector.tensor_add(lgb, lg_ps, bo2)
                nc.sync.dma_start(out=out_d[ds(t, 1)][0], in_=lgb)

                # ---- argmax -> one-hot^T for next gather ----
                mx = wk.tile([BL, 1], f32, tag="mx")
                nc.vector.tensor_reduce(
                    out=mx, in_=lgb, axis=mybir.AxisListType.X, op=mybir.AluOpType.max
                )
                oh = wk.tile([BL, NB], f32, tag="oh")
                nc.vector.tensor_scalar(
                    out=oh, in0=lgb, scalar1=mx, scalar2=None,
                    op0=mybir.AluOpType.is_ge,
                )
                nc.tensor.transpose(oht_ps, oh, id128[0:32, 0:32])
                nc.vector.tensor_copy(oht, oht_ps)

    nc.compile()
    return nc


def _prep_core(I, core):
    """Host-side layout prep for one core's shard (batch rows 32c..32c+32)."""
    sl = slice(core * BL, (core + 1) * BL)
    cf = np.asarray(I["context_features"], np.float32)[sl]  # (32,T,512)
    bh = np.asarray(I["beam_history"])[sl].astype(np.int64)
    be = np.asarray(I["beam_embed"], np.float32)
    W_in = np.asarray(I["W_in"], np.float32)
    b_in = np.asarray(I["b_in"], np.float32)
    W_init = np.asarray(I["W_init"], np.float32)
    b_init = np.asarray(I["b_init"], np.float32)
    W_ih = np.asarray(I["W_ih"], np.float32)
    b_ih = np.asarray(I["b_ih"], np.float32)
    W_hh = np.asarray(I["W_hh"], np.float32)
    b_hh = np.asarray(I["b_hh"], np.float32)
    W_o1 = np.asarray(I["W_o1"], np.float32)
    b_o1 = np.asarray(I["b_o1"], np.float32)
    W_o2 = np.asarray(I["W_o2"], np.float32)
    b_o2 = np.asarray(I["b_o2"], np.float32)
    ln_g = np.asarray(I["ln_g"], np.float32)
    ln_b = np.asarray(I["ln_b"], np.float32)

    # hoisted ctx GEMMs
    A = cf @ W_in[:, :D].T  # (32,T,H)
    Wo1a = W_o1[:, :H]
    C = cf @ W_o1[:, H:].T + b_o1 + ln_b @ Wo1a.T  # (32,T,H)
    at = np.ascontiguousarray(
        A.transpose(1, 2, 0).reshape(T, KT, 128, BL).transpose(0, 2, 1, 3).reshape(T, 128, KT * BL)
    ).astype(np.float32)  # (T,128,256)

    def pack(v):  # (..., T?, H) row-major batch -> packed (128, Q)
        # v (32, H) -> (128, Q): out[32q+b, j] = v[b, 256q+j]
        return np.ascontiguousarray(
            v.reshape(BL, 4, Q).transpose(1, 0, 2).reshape(128, Q)
        )

    cpk = np.ascontiguousarray(
        C.transpose(1, 0, 2).reshape(T, BL, 4, Q).transpose(0, 2, 1, 3).reshape(T, 128, Q)
    ).astype(np.float32)

    # one-time init on host
    prev0 = bh[:, -1]
    hist = be[bh].mean(1)
    ctxg = cf.mean(1)
    h0 = np.tanh(np.concatenate([ctxg, hist], -1) @ W_init.T + b_init).astype(np.float32)
    h0pk = pack(h0)
    h0t = np.ascontiguousarray(
        h0.reshape(BL, 4, 2, 128).transpose(3, 2, 1, 0).reshape(128, 256)
    ).astype(BF16)
    oh0 = np.zeros((NB, BL), np.float32)
    oh0[prev0, np.arange(BL)] = 1.0
    e2 = (be @ W_in[:, D:].T + b_in).astype(BF16)

    def pack_w(wt_cols):  # (H, ncols) W.T columns -> (128, KT, 4, ncols//4)
        n4 = wt_cols.shape[1] // 4
        return np.ascontiguousarray(
            wt_cols.reshape(KT, 128, 4, n4).transpose(1, 0, 2, 3)
        )

    def rz_pack(W):  # (3H, H) -> (128, KT, 4, 512) for r,z quarters
        WT = np.ascontiguousarray(W.T)  # (H, 3H)
        r = WT[:, 0:H].reshape(H, 4, Q)
        z = WT[:, H : 2 * H].reshape(H, 4, Q)
        rz = np.concatenate([r, z], axis=-1).reshape(H, 4 * 2 * Q)
        # [c, q*512+j] -> want [c, q, j]: currently (H, 4, 512) flattened
        return pack_w(rz).astype(BF16)  # (128, KT, 4, 512)

    def n_pack(W):  # (3H, H) -> (128, KT, 4, 256) for n quarters
        WT = np.ascontiguousarray(W.T)
        nn = WT[:, 2 * H :].reshape(H, H)
        return pack_w(nn).astype(BF16)

    Wg = Wo1a * ln_g[None, :]  # scale contraction cols by gamma
    u = Wg.sum(axis=1)  # (H,) rowsums
    wo1 = pack_w(np.ascontiguousarray(Wg.T)).astype(BF16)  # (128, KT, 4, 256)
    u_pk = np.ascontiguousarray(
        np.broadcast_to(u.reshape(4, 1, Q), (4, BL, Q)).reshape(128, Q)
    ).astype(np.float32)
    wo2 = np.ascontiguousarray(W_o2.T.reshape(KT, 128, NB).transpose(1, 0, 2)).astype(BF16)

    def bias_pack(v, width):  # (4*width,) -> (128, width) bcast over batch
        return np.ascontiguousarray(
            np.broadcast_to(v.reshape(4, 1, width), (4, BL, width)).reshape(128, width)
        ).astype(np.float32)

    brz_v = (b_ih + b_hh)[: 2 * H]
    brz = np.concatenate(
        [brz_v[:H].reshape(4, Q), brz_v[H:].reshape(4, Q)], axis=-1
    ).reshape(4 * 2 * Q)
    id128 = np.eye(128, dtype=np.float32)
    cbm = np.zeros((128, 32), np.float32)
    for q in range(4):
        cbm[32 * q + np.arange(32), np.arange(32)] = 1.0
    bbm = np.ascontiguousarray(cbm.T)

    return {
        "wrz_ih": rz_pack(W_ih),
        "wrz_hh": rz_pack(W_hh),
        "wn_ih": n_pack(W_ih),
        "wn_hh": n_pack(W_hh),
        "wo1": wo1,
        "wo2": wo2,
        "e2": e2,
        "upk": u_pk,
        "brz": bias_pack(brz, 2 * Q),
        "bxn": bias_pack(b_ih[2 * H :], Q),
        "bhn": bias_pack(b_hh[2 * H :], Q),
        "bo2": np.ascontiguousarray(np.broadcast_to(b_o2, (BL, NB))).astype(np.float32),
        "id128": id128,
        "cb": cbm,
        "bb": bbm,
        "h0pk": h0pk,
        "h0t": h0t,
        "oh0": oh0.astype(BF16),
        "at": at,
        "cpk": cpk,
    }


def _unshard(res):
    out = np.zeros((B, T, NB), np.float32)
    for c in range(NC):
        out[c * BL : (c + 1) * BL] = res.results[c]["outp"].transpose(1, 0, 2)
    return out


def kernel(**inputs) -> np.ndarray:
    from concourse import bass_utils

    nc = _build()
    in_maps = [_prep_core(inputs, c) for c in range(NC)]
    res = bass_utils.run_bass_kernel_spmd(nc, in_maps, core_ids=list(range(NC)))
    return _unshard(res)
